# revision 28
# baseline (speedup 1.0000x reference)
"""AdaptiveRankLinear on 8 TRN2 NeuronCores.

y[b,t,o] = sum_i x[b,t,i] * W[o,i] + bias[o],  W = U @ (diag(S) @ Vt)

Sharding: pure data-parallel over batch (B=8 == n_cores); U/S/Vt/bias
replicated. Per core: y_b = (x_b @ Vts^T) @ U^T + bias via the rank-256
bottleneck — 2 chained matmuls instead of materializing the 4096x4096 W.

Key design points (v3):
  - x stored as fp8 e3m4: halves x HBM traffic; mm1 runs mixed-dtype
    (bf16 stationary Vts^T, e3m4 moving x) at full PE rate. One e3m4
    quantization hit ~1.2e-2 rel err vs the 2e-2 gate. The DMA byte/
    descriptor reduction also keeps the chip out of the P0 power
    throttle: 8-core matmul sustains 216ns/512cols (2.4GHz) vs 259ns
    (2.0GHz) with bf16 x.
  - two HWDGE rings: x loads on sync (SP), weights + y stores on scalar
    (ACT) — first matmul's operands arrive in parallel; ut loaded in
    need-ordered column slices.
  - 8 dummy K=1 matmuls warm the PE (HAM ramp ~3.4us at 1.2GHz) during
    the initial DMA wait, so real matmuls start at full clock.
  - PE program: mm1(c0) first, bias broadcast matmuls after it (absorb
    the tt-copy latency), then mm1(c+1) segments interleaved BEFORE
    mm2(c) m-blocks so tt copies always hide under mm2 of the previous
    chunk and DVE evacuation work spreads evenly.
  - psum->sbuf evacuation alternates DVE (fused add) and ScalarE copy +
    DVE bf16 add; last chunk evacuates per 512-half right behind the
    accumulation and stores per-oh so the tail is short.
Compute: f32 PSUM accumulate, bf16 output (host casts back to f32).
rel err ~1.2e-2 vs the 2e-2 gate.
"""

import numpy as np
import ml_dtypes

B, T, IN, OUT, RANK = 8, 2048, 4096, 4096, 256
N_CORES = 8
P = 128
TC = 512               # T chunk (psum bank = 512 f32)
NCHUNK = T // TC       # 4
NIT = IN // P          # 32 contraction tiles for mm1
NRT = RANK // P        # 2 rank tiles
OC = 512               # matmul free-dim max
MT = TC // P           # 4 T-tiles per chunk
NG = 4                 # x/vtst load groups per chunk
GN = NIT // NG         # IN tiles per load group (8)
SEG = NIT // MT        # mm1 rows per interleave segment (8)

BF16 = ml_dtypes.bfloat16
E3M4 = ml_dtypes.float8_e3m4

_CACHE = {}


def _build():
    import concourse.bacc as bacc
    import concourse.bass as bass
    import concourse.tile as tile
    from concourse import mybir

    f32 = mybir.dt.float32
    bf16 = mybir.dt.bfloat16
    f8e3 = mybir.dt.float8e3

    nc = bacc.Bacc("TRN2", target_bir_lowering=False, debug=False,
                   num_devices=N_CORES)
    # packed layouts (host-prepped): per (chunk, group) x block is
    # [P, GN*TC] e3m4; per group vtst block is [P, GN*RANK] bf16 —
    # contiguous per-partition rows = descriptor-friendly DMA.
    xp = nc.dram_tensor("xp", [NCHUNK * NG * P, GN * TC], f8e3,
                        kind="ExternalInput")
    vp = nc.dram_tensor("vp", [NG * P, GN * RANK], bf16,
                        kind="ExternalInput")
    ut = nc.dram_tensor("ut", [RANK, OUT], bf16, kind="ExternalInput")
    biasb = nc.dram_tensor("biasb", [P, OUT], bf16, kind="ExternalInput")
    out = nc.dram_tensor("out", [T, OUT], bf16, kind="ExternalOutput")

    with tile.TileContext(nc) as tc:
        with (
            tc.tile_pool(name="weights", bufs=1) as wpool,
            tc.tile_pool(name="xin", bufs=12) as xpool,
            tc.tile_pool(name="tt", bufs=3) as tpool,
            tc.tile_pool(name="yout", bufs=4) as ypool,
            tc.tile_pool(name="pt", bufs=1, space=bass.MemorySpace.PSUM) as ptp,
            tc.tile_pool(name="py", bufs=3, space=bass.MemorySpace.PSUM) as pyp,
        ):
            ones_t = wpool.tile([1, P], bf16, tag="ones")
            nc.vector.memset(ones_t[:], 1.0)

            # ---- loads: x on the sync(SP) ring, weights on scalar(ACT),
            # both FIFO in need-order, so mm1's first operands arrive in
            # parallel ~1.5us after triggers.
            def load_x_group(c, g, parts=1):
                xg = xpool.tile([P, GN * TC], f8e3, tag="xg",
                                name=f"xg_{c}_{g}")
                r0 = (c * NG + g) * P
                w = GN * TC // parts
                for hh in range(parts):
                    nc.sync.dma_start(xg[:, hh * w:(hh + 1) * w],
                                      xp[r0:r0 + P, hh * w:(hh + 1) * w])
                return xg

            vtst_g = []
            for g in range(NG):
                vw = wpool.tile([P, GN * RANK], bf16, tag=f"vtst{g}",
                                name=f"vtst{g}")
                parts = 4 if g == 0 else (2 if g == 1 else 1)
                wv = GN * RANK // parts
                for hh in range(parts):
                    nc.scalar.dma_start(vw[:, hh * wv:(hh + 1) * wv],
                                        vp[g * P:(g + 1) * P,
                                           hh * wv:(hh + 1) * wv])
                vtst_g.append(vw)

            xc = {}
            xc[(0, 0)] = load_x_group(0, 0, parts=4)
            xc[(0, 1)] = load_x_group(0, 1, parts=2)
            for g in range(2, NG):
                xc[(0, g)] = load_x_group(0, g)

            # ut in column slices ordered by mm2's oh need-order
            ut_sb = [wpool.tile([P, OUT], bf16, tag=f"ut{j}", name=f"ut{j}")
                     for j in range(NRT)]
            for o0, o1 in ((0, 2048), (2048, OUT)):
                for j in range(NRT):
                    nc.scalar.dma_start(ut_sb[j][:, o0:o1],
                                        ut[j * P:(j + 1) * P, o0:o1])

            # bias pre-broadcast host-side; needed first at ~29us. Rides
            # the sync ring (slack after chunk-1 x) — the ACT ring is
            # need-tight with vtst+ut through ~31us.
            bias_sb = wpool.tile([P, OUT], bf16, tag="bias")
            xc[(1, 0)] = load_x_group(1, 0)
            xc[(1, 1)] = load_x_group(1, 1)
            nc.sync.dma_start(bias_sb[:], biasb[:, :])
            xc[(1, 2)] = load_x_group(1, 2)
            xc[(1, 3)] = load_x_group(1, 3)
            for c in range(2, NCHUNK):
                for g in range(NG):
                    xc[(c, g)] = load_x_group(c, g)

            tts = {}

            # ---- dummy K=1 matmuls warm the PE HAM (~3.4us at 1.2GHz)
            # during the DMA wait: first operands land ~12.5us (cold-DMA
            # ramp), dummies run ~7.6-12.4 so real mm1 starts at 2.4GHz.
            pd = pyp.tile([P, 1024], f32, tag="py", name="warm")
            for _ in range(44):
                nc.tensor.matmul(pd[:, :P], ones_t[:, :], ones_t[:, :],
                                 start=True, stop=True)

            def emit_mm1(c, n0, n1):
                if n0 == 0:
                    tts[c] = {"pt": [
                        ptp.tile([P, TC], f32, tag=f"pt{j}", name=f"pt{j}_{c}")
                        for j in range(NRT)]}
                pt = tts[c]["pt"]
                for n in range(n0, n1):
                    g, nl = divmod(n, GN)
                    for j in range(NRT):
                        nc.tensor.matmul(
                            pt[j][:],
                            vtst_g[g][:, nl * RANK + j * P:
                                      nl * RANK + (j + 1) * P],
                            xc[(c, g)][:, nl * TC:(nl + 1) * TC],
                            start=(n == 0), stop=(n == NIT - 1))
                if n1 == NIT:
                    tts[c]["tt"] = []
                    for j in range(NRT):
                        ttj = tpool.tile([P, TC], bf16, tag=f"tt{j}",
                                         name=f"tt{j}_{c}")
                        nc.vector.tensor_copy(ttj[:], pt[j][:])
                        tts[c]["tt"].append(ttj)

            def evac(y, py, oh, o0, o1, use_scalar):
                # psum[ :, o0-oh*1024 : o1-... ] -> y[:, o0:o1] (+bias)
                ys = y[:, o0:o1]
                ps = py[:, o0 - oh * 1024:o1 - oh * 1024]
                bs = bias_sb[:, o0:o1]
                if use_scalar:
                    nc.scalar.copy(ys, ps)
                    nc.vector.tensor_add(ys, ys, bs)
                else:
                    nc.vector.tensor_add(ys, ps, bs)

            def emit_mm2_block(c, m):
                tt = tts[c]["tt"]
                last_c = c == NCHUNK - 1
                last_m = last_c and m >= MT - 2
                row = (c * MT + m) * P
                y = ypool.tile([P, OUT], bf16, tag="y")
                for oh in range(OUT // 1024):
                    py = pyp.tile([P, 1024], f32, tag="py")
                    for oo in range(2):
                        for j in range(NRT):
                            o0 = oh * 1024 + oo * OC
                            nc.tensor.matmul(
                                py[:, oo * OC:(oo + 1) * OC],
                                tt[j][:, m * P:(m + 1) * P],
                                ut_sb[j][:, o0:o0 + OC],
                                start=(j == 0), stop=(j == NRT - 1))
                    evac(y, py, oh, oh * 1024, (oh + 1) * 1024,
                         use_scalar=(m * 4 + oh) % 2 == 0)
                    if last_m:
                        # final two tiles: store per-oh, alternating rings
                        # so the tail drains in parallel
                        eng = nc.scalar if (m + oh) % 2 == 0 else nc.sync
                        eng.dma_start(
                            out[row:row + P, oh * 1024:(oh + 1) * 1024],
                            y[:, oh * 1024:(oh + 1) * 1024])
                if not last_m:
                    # alternate store rings per m-tile (sync is idle after
                    # the x loads drain)
                    eng = nc.scalar if (c * MT + m) % 2 == 0 else nc.sync
                    eng.dma_start(out[row:row + P, :], y[:])

            # ---- PE program ----
            emit_mm1(0, 0, NIT)
            for c in range(NCHUNK):
                for m in range(MT):
                    if c + 1 < NCHUNK:
                        emit_mm1(c + 1, m * SEG, (m + 1) * SEG)
                    emit_mm2_block(c, m)

    nc.compile()
    return nc


def _prep_in_maps(x, U, S, Vt, bias):
    x = np.asarray(x, dtype=np.float32)
    U = np.asarray(U, dtype=np.float32)
    S = np.asarray(S, dtype=np.float32)
    Vt = np.asarray(Vt, dtype=np.float32)
    bias = np.asarray(bias, dtype=np.float32)

    vtstT = np.ascontiguousarray((S[:, None] * Vt).T).astype(BF16)  # [IN,R]
    v4 = np.asarray(vtstT).reshape(NIT, P, RANK)
    vp_np = np.concatenate(
        [v4[g * GN:(g + 1) * GN].transpose(1, 0, 2).reshape(P, GN * RANK)
         for g in range(NG)], axis=0)                              # [NG*P, GN*R]
    ut_np = np.ascontiguousarray(U.T).astype(BF16)                 # [R,OUT]
    biasb_np = np.ascontiguousarray(
        np.broadcast_to(bias[None, :].astype(BF16), (P, OUT)))     # [P,OUT]

    in_maps = []
    for c in range(N_CORES):
        xT = np.ascontiguousarray(x[c].T).astype(E3M4)             # [IN,T]
        x4 = xT.reshape(NIT, P, T)
        blocks = []
        for cc in range(NCHUNK):
            for g in range(NG):
                blocks.append(
                    x4[g * GN:(g + 1) * GN, :, cc * TC:(cc + 1) * TC]
                    .transpose(1, 0, 2).reshape(P, GN * TC))
        xp_np = np.concatenate(blocks, axis=0)        # [NCHUNK*NG*P, GN*TC]
        in_maps.append({"xp": xp_np, "vp": vp_np, "ut": ut_np,
                        "biasb": biasb_np})
    return in_maps


def _run(inputs, trace=False, trace_kwargs=None):
    import concourse.bass_utils as bass_utils
    if trace:
        bass_utils.upload_artifacts = lambda tmpdir: tmpdir
    if "nc" not in _CACHE:
        _CACHE["nc"] = _build()
    nc = _CACHE["nc"]
    in_maps = _prep_in_maps(**inputs)
    res = bass_utils.run_bass_kernel_spmd(
        nc, in_maps, core_ids=list(range(N_CORES)), trace=trace,
        **(trace_kwargs or {}))
    y = np.stack([res.results[c]["out"] for c in range(N_CORES)],
                 axis=0).astype(np.float32)
    return y, res


def kernel(**inputs) -> np.ndarray:
    y, _ = _run(inputs, trace=False)
    return y
